# revision 13
# baseline (speedup 1.0000x reference)
"""GCNConv on 8 Trainium2 NeuronCores.

out = D^-1/2 (A + I) D^-1/2 (x @ W.T + b)

Strategy (dest-bucket sharding, readiness-pipelined gather):
  - algebra: fold all per-node scalings out of the device program.
      host:  xs = dis * x                       (pre-scale rows)
      dev:   g = xs @ W.T                       (fp16 table, replicated)
             acc[d] = sum_{e: row=d} g[col_e]   (gather + reduce)
      host:  out[d] = dis[d]*acc[d] + sb[d]*b,  sb[d] = dis[d]*sum_e dis[col_e]
  - host: append self loops, bucket edges by dest core (6250 dests/core),
    sort dests by (max split count, degree) into windows of 128 (one dest
    per SBUF partition). Each dest's sorted cols split ~evenly into three
    int16-addressable gather regions (cols ascending, so each part is a
    node-range prefix/middle/suffix). Windows are greedily packed into
    gather calls of <= 32 columns (4096 descriptors = SWDGE ring cap).
  - pipelining: each gather call's table operand is sliced to the max row
    it actually reads. Tile tracks DRAM deps at byte-range granularity, so
    part-1 calls (cols in the first ~third of nodes) fire as soon as the
    matmul has produced that prefix of the table — the Pool engine's
    descriptor generation (the phase-B critical path) overlaps phase A.
  - phase A: per 4-tile group, one PSUM bank accumulates two matmul passes
    (wt0 then wt1, minimizing weight reloads), one DVE cast fp32->fp16,
    one batched 512-row table write (x loads on the sync queue, table
    writes on the scalar queue).
  - phase B: one DVE tensor_reduce per (window, part) accumulated into a
    per-window fp32 accumulator tile; acc written straight to DRAM.

g table layout (GT_ROWS = NPAD + 384), MIDSPLIT = 24576 is 512-aligned so
matmul groups never straddle the middle zero band:
  row 0..127            zeros
  row n+128             g[node n]          for n < 24576
  row 24704..24831      zeros
  row n+256             g[node n]          for n >= 24576
  row 50304..50431      zeros
Gather regions (int16 idx = row - base):
  G1 base 0     -> nodes <= 32511   (pad idx 0)
  G2 base 8832  -> nodes in [8704, 41343]   (pad idx 15872)
  G3 base 17664 -> nodes >= 17536   (pad idx 32640)
"""

import numpy as np

N_NODES = 50000
N_EDGES = 1600000
IN_CH = 256
OUT_CH = 128
N_CORES = 8

DPC = N_NODES // N_CORES          # dests per core
WPC = (DPC + 127) // 128          # windows per core (49)
NPAD = ((N_NODES + 127) // 128) * 128   # 50048
NT = NPAD // 128                  # node tiles (391)
GT_ROWS = NPAD + 384
MIDSPLIT = 24576                  # 512-aligned middle zero band
GBASE = (0, 8832, 17664)
GPAD = (0, 24704 - 8832, 50304 - 17664)
# group-aligned region bounds (the group-permuted row layout means a whole
# 512-node group must fit the 32768-row int16 window)
G1MAX = 32255
G2MIN, G2MAX = 8704, 40959
G3MIN = 17920

XSLAB_T = 24                      # node tiles per x slab
PGROUP = 4                        # node tiles per PSUM bank / table write
CALLCOLS = 32                     # gather-call column cap (4096 descs = ring)


def _row_of(n):
    """Node -> table row. Groups of PGROUP tiles are internally permuted
    partition-major (row = group_base + p*gn + tile_in_group) so each SBUF
    partition's rows are contiguous in DRAM -> 1KB write descriptors."""
    n = np.asarray(n, dtype=np.int64)
    t = n // 128
    p = n % 128
    g = t // PGROUP
    t0 = g * PGROUP
    gn = np.minimum(PGROUP, NT - t0)
    return 128 + 128 * (n >= MIDSPLIT) + t0 * 128 + p * gn + (t - t0)


def _plan(edge_index):
    """Host-side preprocessing: per-core gather grids (region-major),
    window/call metadata, and the unshard permutation + output scalings."""
    ei0 = np.asarray(edge_index[0], dtype=np.int64)
    ei1 = np.asarray(edge_index[1], dtype=np.int64)
    self_idx = np.arange(N_NODES, dtype=np.int64)
    row = np.concatenate([ei0, self_idx])
    col = np.concatenate([ei1, self_idx])

    deg = np.bincount(row, minlength=N_NODES)
    dis = deg.astype(np.float64) ** -0.5
    ssum = np.bincount(row, weights=dis[col], minlength=N_NODES)
    sb = (dis * ssum).astype(np.float32)

    # 3-way split of each dest's ascending cols: first k1 -> G1, middle k2
    # -> G2, last k3 -> G3; clip keeps every part region-valid.
    n1o = np.bincount(row[col < G2MIN], minlength=N_NODES)
    n3o = np.bincount(row[col > G2MAX], minlength=N_NODES)
    m1 = np.bincount(row[col <= G1MAX], minlength=N_NODES)
    m3 = np.bincount(row[col >= G3MIN], minlength=N_NODES)
    k1 = np.clip((deg + 2) // 3, n1o, m1)
    k3 = np.clip((deg - k1 + 1) // 2, n3o, np.minimum(m3, deg - k1))
    k2 = deg - k1 - k3
    k = np.stack([k1, k2, k3])
    roff = np.stack([np.zeros_like(k1), k1, k1 + k2])
    maxk = k.max(axis=0)

    order = np.lexsort((col, row))
    rows_sorted = _row_of(col[order])
    starts = np.zeros(N_NODES + 1, dtype=np.int64)
    np.cumsum(deg, out=starts[1:])

    perms = []
    kw = np.zeros((3, N_CORES, WPC), dtype=np.int64)
    for c in range(N_CORES):
        sl = slice(c * DPC, (c + 1) * DPC)
        perm = np.lexsort((-deg[sl], -maxk[sl]))
        perms.append(perm)
        for r in range(3):
            ks = np.pad(k[r, sl][perm], (0, WPC * 128 - DPC))
            kw[r, c] = ks.reshape(WPC, 128).max(axis=1)
    cks = kw.max(axis=1)              # [3, WPC] global per-window maxima
    cks = np.maximum(cks, 1)

    offs = np.zeros((3, WPC), dtype=np.int64)   # col offset within segment
    batches = []                      # [3][nbatch] -> list of windows
    for r in range(3):
        offs[r] = np.concatenate([[0], np.cumsum(cks[r])[:-1]])
        bs, cur, acc = [], [], 0
        for w in range(WPC):
            kc = int(cks[r, w])
            if cur and acc + kc > CALLCOLS:
                bs.append(cur)
                cur, acc = [], 0
            cur.append(w)
            acc += kc
        if cur:
            bs.append(cur)
        batches.append(bs)
    Ls = cks.sum(axis=1)
    seg0 = np.concatenate([[0], np.cumsum(Ls)[:-1]])
    totc = int(Ls.sum())

    idx16, gdests = [], []
    grids = []
    for c in range(N_CORES):
        perm = perms[c]
        gdest = np.full((WPC, 128), -1, dtype=np.int64)
        dest_all = np.full(WPC * 128, -1, dtype=np.int64)
        dest_all[:DPC] = c * DPC + perm
        gdest[:] = dest_all.reshape(WPC, 128)
        valid = dest_all >= 0
        d = dest_all[valid]
        p = (np.arange(WPC * 128) % 128)[valid]
        wi = (np.arange(WPC * 128) // 128)[valid]
        subs = []
        for r in range(3):
            sub = np.full((128, int(Ls[r])), GPAD[r], dtype=np.int16)
            kk = k[r, d]
            tot = int(kk.sum())
            if tot:
                intra = np.arange(tot) - np.repeat(
                    np.concatenate([[0], np.cumsum(kk)[:-1]]), kk)
                tgt = np.repeat(p * Ls[r] + offs[r, wi], kk) + intra
                src = np.repeat(starts[d] + roff[r, d], kk) + intra
                vals = rows_sorted[src] - GBASE[r]
                assert vals.min() >= 0 and vals.max() <= 32767, (
                    r, vals.min(), vals.max())
                sub.ravel()[tgt] = vals.astype(np.int16)
            subs.append(sub)
        grid = np.concatenate(subs, axis=1)
        grids.append(grid)
        # dma_gather idx layout: slot j (= col*128 + part) at
        # [16g + j%16, j//16] for each of 8 replicated 16-row groups.
        L = grid.T.ravel()
        base = L.reshape(totc * 8, 16).T
        idx16.append(np.ascontiguousarray(np.tile(base, (8, 1))))
        gdests.append(gdest)

    # per-call max table row read (over all cores -> one shared NEFF);
    # the sliced gather operand makes Tile start the call as soon as the
    # matmul has written that prefix of the table.
    maxrows = []
    for r in range(3):
        mr = []
        for batch in batches[r]:
            o = int(seg0[r] + offs[r, batch[0]])
            bc = int(sum(cks[r, w] for w in batch))
            m = 0
            for c in range(N_CORES):
                m = max(m, int(grids[c][:, o:o + bc].max()))
            mr.append(GBASE[r] + m + 1)
        maxrows.append(mr)

    return {
        "dis": dis.astype(np.float32),
        "sb": sb,
        "cks": cks,
        "offs": offs,
        "seg0": seg0,
        "totc": totc,
        "batches": batches,
        "maxrows": maxrows,
        "idx16": idx16,
        "gdest": gdests,
    }


def _build_bass(cks, offs, seg0, totc, batches, maxrows):
    import concourse.bacc as bacc
    import concourse.mybir as mybir
    import concourse.tile as tile
    from concourse.library_config import mlp

    fp32 = mybir.dt.float32
    fp16 = mybir.dt.float16
    i16 = mybir.dt.int16

    nc = bacc.Bacc(
        "TRN2",
        target_bir_lowering=False,
        dynamic_dma_scratch_size=131072,
        num_swdge_queues=4,
    )

    xT = nc.dram_tensor("xT", [IN_CH, NPAD], fp16, kind="ExternalInput")
    wT = nc.dram_tensor("wT", [IN_CH, OUT_CH], fp16, kind="ExternalInput")
    idx = nc.dram_tensor("idx", [128, totc * 8], i16, kind="ExternalInput")

    gtab = nc.dram_tensor("gtab", [GT_ROWS, OUT_CH], fp16, kind="Internal")
    outd = nc.dram_tensor("outd", [WPC, 128, OUT_CH], fp32,
                          kind="ExternalOutput")

    with tile.TileContext(nc) as tc:
        with tc.tile_pool(name="constB", bufs=1) as bpool:
            nc.gpsimd.load_library(mlp)
            ix = bpool.tile([128, totc * 8], i16, tag="ix")
            nc.scalar.dma_start(ix[:], idx[:])

            # ------------- phase A: g = xs @ W.T (pre-scaled x) -------------
            with (
                tc.tile_pool(name="constA", bufs=1) as cpool,
                tc.tile_pool(name="xslab", bufs=2) as xpool,
                tc.tile_pool(name="gout", bufs=4) as gpool,
                tc.tile_pool(name="psum", bufs=2, space="PSUM") as ppool,
            ):
                wt0 = cpool.tile([128, OUT_CH], fp16, tag="wt0")
                wt1 = cpool.tile([128, OUT_CH], fp16, tag="wt1")
                nc.sync.dma_start(wt0[:], wT[0:128, :])
                nc.sync.dma_start(wt1[:], wT[128:256, :])

                # zero bands absorb padding gathers
                zt = cpool.tile([128, OUT_CH], fp16, tag="zt")
                nc.vector.memset(zt[:], 0.0)
                for r0 in (0, 24704, 50304):
                    nc.scalar.dma_start(gtab[r0:r0 + 128, :], zt[:])

                nslab = (NT + XSLAB_T - 1) // XSLAB_T
                for s in range(nslab):
                    t0 = s * XSLAB_T
                    ntile = min(XSLAB_T, NT - t0)
                    cols = ntile * 128
                    xa = xpool.tile([128, XSLAB_T * 128], fp16, tag="xa")
                    xb = xpool.tile([128, XSLAB_T * 128], fp16, tag="xb")
                    c0 = t0 * 128
                    nc.sync.dma_start(xa[:, 0:cols], xT[0:128, c0:c0 + cols])
                    nc.sync.dma_start(xb[:, 0:cols], xT[128:256, c0:c0 + cols])
                    t = 0
                    while t < ntile:
                        gn = min(PGROUP, ntile - t)
                        # separate PSUM tiles: accumulation groups may not
                        # interleave within one bank (only the last start/
                        # stop pair survives), but do across banks.
                        pss = []
                        for j in range(gn):
                            psj = ppool.tile([128, OUT_CH], fp32,
                                             tag=f"ps{j}", name=f"ps_{j}")
                            pss.append(psj)
                        gtl = gpool.tile([128, PGROUP, OUT_CH], fp16, tag="gt")
                        for j in range(gn):
                            sl = slice((t + j) * 128, (t + j + 1) * 128)
                            nc.tensor.matmul(
                                pss[j][:], xa[:, sl], wt0[:],
                                start=True, stop=False,
                            )
                        for j in range(gn):
                            sl = slice((t + j) * 128, (t + j + 1) * 128)
                            nc.tensor.matmul(
                                pss[j][:], xb[:, sl], wt1[:],
                                start=False, stop=True,
                            )
                        for j in range(gn):
                            nc.vector.tensor_scalar_mul(
                                gtl[:, j, :], pss[j][:], 1.0
                            )
                        node0 = (t0 + t) * 128
                        w0 = node0 + 128 + 128 * (node0 >= MIDSPLIT)
                        dst = gtab[w0:w0 + gn * 128, :].rearrange(
                            "(p a) c -> p a c", a=gn)
                        nc.scalar.dma_start(dst, gtl[:, 0:gn, :])
                        t += gn

            # ------------- phase B: gather + segment reduce -----------------
            with (
                tc.tile_pool(name="msg", bufs=2) as mpool,
                tc.tile_pool(name="accp", bufs=1) as apool,
                tc.tile_pool(name="red", bufs=4) as rpool,
            ):
                acc = apool.tile([128, WPC, OUT_CH], fp32, tag="acc")
                gq = 0
                for r in range(3):
                    # fire calls in table-readiness order
                    order = np.argsort(maxrows[r], kind="stable")
                    for bi in order:
                        batch = batches[r][bi]
                        bc = int(sum(cks[r, w] for w in batch))
                        o = int(seg0[r] + offs[r, batch[0]])
                        src = gtab[GBASE[r]:maxrows[r][bi], :]
                        msg = mpool.tile([128, CALLCOLS, OUT_CH], fp16,
                                         tag="msg")
                        nc.gpsimd.dma_gather(
                            msg[:, 0:bc, :], src,
                            ix[:, o * 8:(o + bc) * 8],
                            128 * bc, 128 * bc, OUT_CH,
                            queue_num=gq % 4,
                            single_packet=False,
                        )
                        gq += 1
                        coff = 0
                        for w in batch:
                            kc = int(cks[r, w])
                            red_in = msg[:, coff:coff + kc, :].transpose(
                                [0, 2, 1])
                            if r == 0:
                                nc.vector.tensor_reduce(
                                    acc[:, w, :], red_in,
                                    axis=mybir.AxisListType.X,
                                    op=mybir.AluOpType.add,
                                )
                            else:
                                rt = rpool.tile([128, OUT_CH], fp32, tag="rt")
                                nc.vector.tensor_reduce(
                                    rt[:], red_in,
                                    axis=mybir.AxisListType.X,
                                    op=mybir.AluOpType.add,
                                )
                                nc.vector.tensor_tensor(
                                    acc[:, w, :], acc[:, w, :], rt[:],
                                    op=mybir.AluOpType.add,
                                )
                                if r == 2:
                                    nc.sync.dma_start(outd[w], acc[:, w, :])
                            coff += kc

    nc.compile()
    return nc


def _install_ntff_shim():
    """The agent image's antenv lacks axon_hooks; register a shim wired to
    the libaxon NTFF profiler so trace=True works."""
    import sys
    import types
    try:
        import antenv.axon_hooks  # noqa: F401
        return
    except ImportError:
        pass
    hook = None
    try:
        from trn_agent_boot.trn_boot import _ntff_profile_via_ctypes
        hook = _ntff_profile_via_ctypes("/opt/axon/libaxon_pjrt.so")
    except Exception:
        hook = None
    mod = types.ModuleType("antenv.axon_hooks")
    mod._hook = hook
    mod.get_axon_ntff_profile_hook = lambda: mod._hook
    def _set(h):
        mod._hook = h
    mod.set_axon_ntff_profile_hook = _set
    sys.modules["antenv.axon_hooks"] = mod
    try:
        import antenv
        antenv.axon_hooks = mod
    except Exception:
        pass


def kernel(x, edge_index, W, b):
    import os
    os.environ.setdefault("NEURON_RT_RESET_CORES", "1")
    x = np.asarray(x, dtype=np.float32)
    W = np.asarray(W, dtype=np.float32)
    b = np.asarray(b, dtype=np.float32)

    plan = _plan(edge_index)
    nc = _build_bass(plan["cks"], plan["offs"], plan["seg0"], plan["totc"],
                     plan["batches"], plan["maxrows"])

    dis = plan["dis"]
    xs = x * dis[:, None]
    xT_pad = np.zeros((IN_CH, NPAD), dtype=np.float16)
    xT_pad[:, :N_NODES] = xs.T.astype(np.float16)
    wT = np.ascontiguousarray(W.T.astype(np.float16))

    in_maps = []
    for c in range(N_CORES):
        in_maps.append({
            "xT": xT_pad,
            "wT": wT,
            "idx": plan["idx16"][c],
        })

    _install_ntff_shim()
    from concourse.bass_utils import run_bass_kernel_spmd
    res = run_bass_kernel_spmd(nc, in_maps, core_ids=list(range(N_CORES)))
    globals()["_last_results"] = res

    out = np.empty((N_NODES, OUT_CH), dtype=np.float32)
    for c in range(N_CORES):
        outd = res.results[c]["outd"]
        gdest = plan["gdest"][c]
        mask = gdest >= 0
        out[gdest[mask]] = outd[mask]
    out *= dis[:, None]
    out += plan["sb"][:, None] * b[None, :]
    return out


# revision 20
# speedup vs baseline: 2.0328x; 2.0328x over previous
"""GCNConv on 8 Trainium2 NeuronCores.

out = D^-1/2 (A + I) D^-1/2 (x @ W.T + b)

Strategy (dest-bucket sharding, readiness-pipelined gather):
  - algebra: fold all per-node scalings out of the device program.
      host:  xs = dis * x                       (pre-scale rows)
      dev:   g = xs @ W.T                       (fp16 table, replicated)
             acc[d] = sum_{e: row=d} g[col_e]   (gather + reduce)
      host:  out[d] = dis[d]*acc[d] + sb[d]*b,  sb[d] = dis[d]*sum_e dis[col_e]
  - host: append self loops, bucket edges by dest core (6250 dests/core),
    sort dests by (max split count, degree) into windows of 128 (one dest
    per SBUF partition). Each dest's sorted cols split ~evenly into three
    int16-addressable gather regions (cols ascending, so each part is a
    node-range prefix/middle/suffix). Windows are greedily packed into
    gather calls of <= 32 columns (4096 descriptors = SWDGE ring cap).
  - pipelining: each gather call's table operand is sliced to the max row
    it actually reads. Tile tracks DRAM deps at byte-range granularity, so
    part-1 calls (cols in the first ~third of nodes) fire as soon as the
    matmul has produced that prefix of the table — the Pool engine's
    descriptor generation (the phase-B critical path) overlaps phase A.
  - phase A: per 4-tile group, one PSUM bank accumulates two matmul passes
    (wt0 then wt1, minimizing weight reloads), one DVE cast fp32->fp16,
    one batched 512-row table write (x loads on the sync queue, table
    writes on the scalar queue).
  - phase B: one DVE tensor_reduce per (window, part) accumulated into a
    per-window fp32 accumulator tile; acc written straight to DRAM.

g table layout (GT_ROWS = NPAD + 384), MIDSPLIT = 24576 is 512-aligned so
matmul groups never straddle the middle zero band:
  row 0..127            zeros
  row n+128             g[node n]          for n < 24576
  row 24704..24831      zeros
  row n+256             g[node n]          for n >= 24576
  row 50304..50431      zeros
Gather regions (int16 idx = row - base):
  G1 base 0     -> nodes <= 32511   (pad idx 0)
  G2 base 8832  -> nodes in [8704, 41343]   (pad idx 15872)
  G3 base 17664 -> nodes >= 17536   (pad idx 32640)
"""

import numpy as np

N_NODES = 50000
N_EDGES = 1600000
IN_CH = 256
OUT_CH = 128
N_CORES = 8

DPC = N_NODES // N_CORES          # dests per core
WPC = (DPC + 127) // 128          # windows per core (49)
NPAD = ((N_NODES + 127) // 128) * 128   # 50048
NT = NPAD // 128                  # node tiles (391)
GT_ROWS = NPAD + 384
MIDSPLIT = 24576                  # 512-aligned middle zero band
GBASE = (0, 8832, 17664)
GPAD = (0, 24704 - 8832, 50304 - 17664)
# group-aligned region bounds (the group-permuted row layout means a whole
# 512-node group must fit the 32768-row int16 window)
G1MAX = 32255
G2MIN, G2MAX = 8704, 40959
G3MIN = 17920

XSLAB_T = 24                      # node tiles per x slab
PGROUP = 4                        # node tiles per PSUM bank / table write
CALLCOLS = 16                     # gather-call column cap (2048 descs; half
                                  # the SWDGE ring, so desc-gen of the next
                                  # call overlaps the prior call's transfer)
MSGCOLS = 32                      # window-batch column budget (msg tile)


def _row_of(n):
    """Node -> table row. Groups of PGROUP tiles are internally permuted
    partition-major (row = group_base + p*gn + tile_in_group) so each SBUF
    partition's rows are contiguous in DRAM -> 1KB write descriptors."""
    n = np.asarray(n, dtype=np.int64)
    t = n // 128
    p = n % 128
    g = t // PGROUP
    t0 = g * PGROUP
    gn = np.minimum(PGROUP, NT - t0)
    return 128 + 128 * (n >= MIDSPLIT) + t0 * 128 + p * gn + (t - t0)


def _plan(edge_index):
    """Host-side preprocessing: per-core gather grids (region-major),
    window/call metadata, and the unshard permutation + output scalings."""
    ei0 = np.asarray(edge_index[0], dtype=np.int64)
    ei1 = np.asarray(edge_index[1], dtype=np.int64)
    self_idx = np.arange(N_NODES, dtype=np.int64)
    row = np.concatenate([ei0, self_idx])
    col = np.concatenate([ei1, self_idx])

    deg = np.bincount(row, minlength=N_NODES)
    dis = deg.astype(np.float64) ** -0.5
    ssum = np.bincount(row, weights=dis[col], minlength=N_NODES)
    sb = (dis * ssum).astype(np.float32)

    # 3-way split of each dest's ascending cols: first k1 -> G1, middle k2
    # -> G2, last k3 -> G3; clip keeps every part region-valid.
    n1o = np.bincount(row[col < G2MIN], minlength=N_NODES)
    n3o = np.bincount(row[col > G2MAX], minlength=N_NODES)
    m1 = np.bincount(row[col <= G1MAX], minlength=N_NODES)
    m3 = np.bincount(row[col >= G3MIN], minlength=N_NODES)
    k1 = np.clip((deg + 2) // 3, n1o, m1)
    k3 = np.clip((deg - k1 + 1) // 2, n3o, np.minimum(m3, deg - k1))
    k2 = deg - k1 - k3
    k = np.stack([k1, k2, k3])
    roff = np.stack([np.zeros_like(k1), k1, k1 + k2])
    maxk = k.max(axis=0)

    order = np.lexsort((col, row))
    rows_sorted = _row_of(col[order])
    starts = np.zeros(N_NODES + 1, dtype=np.int64)
    np.cumsum(deg, out=starts[1:])

    perms = []
    kw = np.zeros((3, N_CORES, WPC), dtype=np.int64)
    for c in range(N_CORES):
        sl = slice(c * DPC, (c + 1) * DPC)
        perm = np.lexsort((-deg[sl], -maxk[sl]))
        perms.append(perm)
        for r in range(3):
            ks = np.pad(k[r, sl][perm], (0, WPC * 128 - DPC))
            kw[r, c] = ks.reshape(WPC, 128).max(axis=1)
    cks = kw.max(axis=1)              # [3, WPC] global per-window maxima
    cks = np.maximum(cks, 1)

    offs = np.zeros((3, WPC), dtype=np.int64)   # col offset within segment
    batches = []                      # [3][nbatch] -> list of windows
    for r in range(3):
        offs[r] = np.concatenate([[0], np.cumsum(cks[r])[:-1]])
        bs, cur, acc = [], [], 0
        for w in range(WPC):
            kc = int(cks[r, w])
            if cur and acc + kc > MSGCOLS:
                bs.append(cur)
                cur, acc = [], 0
            cur.append(w)
            acc += kc
        if cur:
            bs.append(cur)
        batches.append(bs)
    Ls = cks.sum(axis=1)
    seg0 = np.concatenate([[0], np.cumsum(Ls)[:-1]])
    totc = int(Ls.sum())

    idx16, gdests = [], []
    grids = []
    for c in range(N_CORES):
        perm = perms[c]
        gdest = np.full((WPC, 128), -1, dtype=np.int64)
        dest_all = np.full(WPC * 128, -1, dtype=np.int64)
        dest_all[:DPC] = c * DPC + perm
        gdest[:] = dest_all.reshape(WPC, 128)
        valid = dest_all >= 0
        d = dest_all[valid]
        p = (np.arange(WPC * 128) % 128)[valid]
        wi = (np.arange(WPC * 128) // 128)[valid]
        subs = []
        for r in range(3):
            sub = np.full((128, int(Ls[r])), GPAD[r], dtype=np.int16)
            kk = k[r, d]
            tot = int(kk.sum())
            if tot:
                intra = np.arange(tot) - np.repeat(
                    np.concatenate([[0], np.cumsum(kk)[:-1]]), kk)
                tgt = np.repeat(p * Ls[r] + offs[r, wi], kk) + intra
                src = np.repeat(starts[d] + roff[r, d], kk) + intra
                vals = rows_sorted[src] - GBASE[r]
                assert vals.min() >= 0 and vals.max() <= 32767, (
                    r, vals.min(), vals.max())
                sub.ravel()[tgt] = vals.astype(np.int16)
            subs.append(sub)
        grid = np.concatenate(subs, axis=1)
        grids.append(grid)
        # dma_gather idx layout: slot j (= col*128 + part) at
        # [16g + j%16, j//16] for each of 8 replicated 16-row groups.
        L = grid.T.ravel()
        base = L.reshape(totc * 8, 16).T
        idx16.append(np.ascontiguousarray(np.tile(base, (8, 1))))
        gdests.append(gdest)

    # per-call max table row read (over all cores -> one shared NEFF);
    # the sliced gather operand makes Tile start the call as soon as the
    # matmul has written that prefix of the table.
    maxrows = []
    for r in range(3):
        mr = []
        for batch in batches[r]:
            o = int(seg0[r] + offs[r, batch[0]])
            bc = int(sum(cks[r, w] for w in batch))
            m = 0
            for c in range(N_CORES):
                m = max(m, int(grids[c][:, o:o + bc].max()))
            mr.append(GBASE[r] + m + 1)
        maxrows.append(mr)

    return {
        "dis": dis.astype(np.float32),
        "sb": sb,
        "cks": cks,
        "offs": offs,
        "seg0": seg0,
        "totc": totc,
        "batches": batches,
        "maxrows": maxrows,
        "idx16": idx16,
        "gdest": gdests,
    }


def _build_bass(cks, offs, seg0, totc, batches, maxrows):
    import concourse.bacc as bacc
    import concourse.mybir as mybir
    import concourse.tile as tile
    from concourse.library_config import mlp

    fp32 = mybir.dt.float32
    fp16 = mybir.dt.float16
    i16 = mybir.dt.int16

    nc = bacc.Bacc(
        "TRN2",
        target_bir_lowering=False,
        dynamic_dma_scratch_size=65536,
        num_swdge_queues=4,
    )

    xT = nc.dram_tensor("xT", [IN_CH, NPAD], fp16, kind="ExternalInput")
    wT = nc.dram_tensor("wT", [IN_CH, OUT_CH], fp16, kind="ExternalInput")
    idx = nc.dram_tensor("idx", [128, totc * 8], i16, kind="ExternalInput")

    gtab = nc.dram_tensor("gtab", [GT_ROWS, OUT_CH], fp16, kind="Internal")
    outd = nc.dram_tensor("outd", [WPC, 128, OUT_CH], fp32,
                          kind="ExternalOutput")

    with tile.TileContext(nc) as tc:
        # phase-B pools open first: their SBUF lives below the phase-A
        # pools, so the first gathers don't inherit a WAR dependency on
        # phase A's last instructions through memory reuse.
        with (
            tc.tile_pool(name="constB", bufs=1) as bpool,
            tc.tile_pool(name="msg", bufs=4) as mpool,
            tc.tile_pool(name="accp", bufs=1) as apool,
            tc.tile_pool(name="red", bufs=4) as rpool,
        ):
            nc.gpsimd.load_library(mlp)
            ix = bpool.tile([128, totc * 8], i16, tag="ix")
            nc.scalar.dma_start(ix[:], idx[:])

            # ------------- phase A: g = xs @ W.T (pre-scaled x) -------------
            with (
                tc.tile_pool(name="constA", bufs=1) as cpool,
                tc.tile_pool(name="xslab", bufs=2) as xpool,
                tc.tile_pool(name="gout", bufs=4) as gpool,
                tc.tile_pool(name="psum", bufs=2, space="PSUM") as ppool,
            ):
                wt0 = cpool.tile([128, OUT_CH], fp16, tag="wt0")
                wt1 = cpool.tile([128, OUT_CH], fp16, tag="wt1")
                nc.sync.dma_start(wt0[:], wT[0:128, :])
                nc.sync.dma_start(wt1[:], wT[128:256, :])

                # zero bands absorb padding gathers
                zt = cpool.tile([128, OUT_CH], fp16, tag="zt")
                nc.vector.memset(zt[:], 0.0)
                for r0 in (0, 24704, 50304):
                    nc.scalar.dma_start(gtab[r0:r0 + 128, :], zt[:])

                nslab = (NT + XSLAB_T - 1) // XSLAB_T
                for s in range(nslab):
                    t0 = s * XSLAB_T
                    ntile = min(XSLAB_T, NT - t0)
                    cols = ntile * 128
                    xa = xpool.tile([128, XSLAB_T * 128], fp16, tag="xa")
                    xb = xpool.tile([128, XSLAB_T * 128], fp16, tag="xb")
                    c0 = t0 * 128
                    nc.sync.dma_start(xa[:, 0:cols], xT[0:128, c0:c0 + cols])
                    nc.sync.dma_start(xb[:, 0:cols], xT[128:256, c0:c0 + cols])
                    t = 0
                    while t < ntile:
                        gn = min(PGROUP, ntile - t)
                        # separate PSUM tiles: accumulation groups may not
                        # interleave within one bank (only the last start/
                        # stop pair survives), but do across banks.
                        pss = []
                        for j in range(gn):
                            psj = ppool.tile([128, OUT_CH], fp32,
                                             tag=f"ps{j}", name=f"ps_{j}")
                            pss.append(psj)
                        gtl = gpool.tile([128, PGROUP, OUT_CH], fp16, tag="gt")
                        for j in range(gn):
                            sl = slice((t + j) * 128, (t + j + 1) * 128)
                            nc.tensor.matmul(
                                pss[j][:], xa[:, sl], wt0[:],
                                start=True, stop=False,
                            )
                        for j in range(gn):
                            sl = slice((t + j) * 128, (t + j + 1) * 128)
                            nc.tensor.matmul(
                                pss[j][:], xb[:, sl], wt1[:],
                                start=False, stop=True,
                            )
                        for j in range(gn):
                            nc.vector.tensor_scalar_mul(
                                gtl[:, j, :], pss[j][:], 1.0
                            )
                        node0 = (t0 + t) * 128
                        w0 = node0 + 128 + 128 * (node0 >= MIDSPLIT)
                        dst = gtab[w0:w0 + gn * 128, :].rearrange(
                            "(p a) c -> p a c", a=gn)
                        nc.scalar.dma_start(dst, gtl[:, 0:gn, :])
                        t += gn

            # ------------- phase B: gather + segment reduce -----------------
            if True:
                acc = apool.tile([128, WPC, OUT_CH], fp32, tag="acc")
                gq = 0
                for r in range(3):
                    # fire calls in table-readiness order
                    order = np.argsort(maxrows[r], kind="stable")
                    for bi in order:
                        batch = batches[r][bi]
                        bc = int(sum(cks[r, w] for w in batch))
                        o = int(seg0[r] + offs[r, batch[0]])
                        src = gtab[GBASE[r]:maxrows[r][bi], :]
                        msg = mpool.tile([128, MSGCOLS, OUT_CH], fp16,
                                         tag="msg")
                        for cc0 in range(0, bc, CALLCOLS):
                            cc = min(CALLCOLS, bc - cc0)
                            nc.gpsimd.dma_gather(
                                msg[:, cc0:cc0 + cc, :], src,
                                ix[:, (o + cc0) * 8:(o + cc0 + cc) * 8],
                                128 * cc, 128 * cc, OUT_CH,
                                queue_num=gq % 4,
                                single_packet=False,
                            )
                            gq += 1
                        coff = 0
                        for w in batch:
                            kc = int(cks[r, w])
                            red_in = msg[:, coff:coff + kc, :].transpose(
                                [0, 2, 1])
                            if r == 0:
                                nc.vector.tensor_reduce(
                                    acc[:, w, :], red_in,
                                    axis=mybir.AxisListType.X,
                                    op=mybir.AluOpType.add,
                                )
                            else:
                                rt = rpool.tile([128, OUT_CH], fp32, tag="rt")
                                nc.vector.tensor_reduce(
                                    rt[:], red_in,
                                    axis=mybir.AxisListType.X,
                                    op=mybir.AluOpType.add,
                                )
                                nc.vector.tensor_tensor(
                                    acc[:, w, :], acc[:, w, :], rt[:],
                                    op=mybir.AluOpType.add,
                                )
                                if r == 2:
                                    nc.sync.dma_start(outd[w], acc[:, w, :])
                            coff += kc

    nc.compile()
    return nc


def _install_ntff_shim():
    """The agent image's antenv lacks axon_hooks; register a shim wired to
    the libaxon NTFF profiler so trace=True works."""
    import sys
    import types
    try:
        import antenv.axon_hooks  # noqa: F401
        return
    except ImportError:
        pass
    hook = None
    try:
        from trn_agent_boot.trn_boot import _ntff_profile_via_ctypes
        hook = _ntff_profile_via_ctypes("/opt/axon/libaxon_pjrt.so")
    except Exception:
        hook = None
    mod = types.ModuleType("antenv.axon_hooks")
    mod._hook = hook
    mod.get_axon_ntff_profile_hook = lambda: mod._hook
    def _set(h):
        mod._hook = h
    mod.set_axon_ntff_profile_hook = _set
    sys.modules["antenv.axon_hooks"] = mod
    try:
        import antenv
        antenv.axon_hooks = mod
    except Exception:
        pass


def kernel(x, edge_index, W, b):
    import os
    os.environ.setdefault("NEURON_RT_RESET_CORES", "1")
    x = np.asarray(x, dtype=np.float32)
    W = np.asarray(W, dtype=np.float32)
    b = np.asarray(b, dtype=np.float32)

    plan = _plan(edge_index)
    nc = _build_bass(plan["cks"], plan["offs"], plan["seg0"], plan["totc"],
                     plan["batches"], plan["maxrows"])

    dis = plan["dis"]
    xs = x * dis[:, None]
    xT_pad = np.zeros((IN_CH, NPAD), dtype=np.float16)
    xT_pad[:, :N_NODES] = xs.T.astype(np.float16)
    wT = np.ascontiguousarray(W.T.astype(np.float16))

    in_maps = []
    for c in range(N_CORES):
        in_maps.append({
            "xT": xT_pad,
            "wT": wT,
            "idx": plan["idx16"][c],
        })

    _install_ntff_shim()
    from concourse.bass_utils import run_bass_kernel_spmd
    res = run_bass_kernel_spmd(nc, in_maps, core_ids=list(range(N_CORES)))
    globals()["_last_results"] = res

    out = np.empty((N_NODES, OUT_CH), dtype=np.float32)
    for c in range(N_CORES):
        outd = res.results[c]["outd"]
        gdest = plan["gdest"][c]
        mask = gdest >= 0
        out[gdest[mask]] = outd[mask]
    out *= dis[:, None]
    out += plan["sb"][:, None] * b[None, :]
    return out


# revision 23
# speedup vs baseline: 2.1258x; 1.0457x over previous
"""GCNConv on 8 Trainium2 NeuronCores.

out = D^-1/2 (A + I) D^-1/2 (x @ W.T + b)

Strategy (dest-bucket sharding, readiness-pipelined gather):
  - algebra: fold all per-node scalings out of the device program.
      host:  xs = dis * x                       (pre-scale rows)
      dev:   g = xs @ W.T                       (fp16 table, replicated)
             acc[d] = sum_{e: row=d} g[col_e]   (gather + reduce)
      host:  out[d] = dis[d]*acc[d] + sb[d]*b,  sb[d] = dis[d]*sum_e dis[col_e]
  - host: append self loops, bucket edges by dest core (6250 dests/core),
    sort dests by (max split count, degree) into windows of 128 (one dest
    per SBUF partition). Each dest's sorted cols split ~evenly into three
    int16-addressable gather regions (cols ascending, so each part is a
    node-range prefix/middle/suffix). Windows are greedily packed into
    gather calls of <= 32 columns (4096 descriptors = SWDGE ring cap).
  - pipelining: each gather call's table operand is sliced to the max row
    it actually reads. Tile tracks DRAM deps at byte-range granularity, so
    part-1 calls (cols in the first ~third of nodes) fire as soon as the
    matmul has produced that prefix of the table — the Pool engine's
    descriptor generation (the phase-B critical path) overlaps phase A.
  - phase A: per 4-tile group, one PSUM bank accumulates two matmul passes
    (wt0 then wt1, minimizing weight reloads), one DVE cast fp32->fp16,
    one batched 512-row table write (x loads on the sync queue, table
    writes on the scalar queue).
  - phase B: one DVE tensor_reduce per (window, part) accumulated into a
    per-window fp32 accumulator tile; acc written straight to DRAM.

g table layout (GT_ROWS = NPAD + 384), MIDSPLIT = 24576 is 512-aligned so
matmul groups never straddle the middle zero band:
  row 0..127            zeros
  row n+128             g[node n]          for n < 24576
  row 24704..24831      zeros
  row n+256             g[node n]          for n >= 24576
  row 50304..50431      zeros
Gather regions (int16 idx = row - base):
  G1 base 0     -> nodes <= 32511   (pad idx 0)
  G2 base 8832  -> nodes in [8704, 41343]   (pad idx 15872)
  G3 base 17664 -> nodes >= 17536   (pad idx 32640)
"""

import numpy as np

N_NODES = 50000
N_EDGES = 1600000
IN_CH = 256
OUT_CH = 128
N_CORES = 8

DPC = N_NODES // N_CORES          # dests per core
WPC = (DPC + 127) // 128          # windows per core (49)
NPAD = ((N_NODES + 127) // 128) * 128   # 50048
NT = NPAD // 128                  # node tiles (391)
GT_ROWS = NPAD + 384
MIDSPLIT = 24576                  # 512-aligned middle zero band
GBASE = (0, 8832, 17664)
GPAD = (0, 24704 - 8832, 50304 - 17664)
# group-aligned region bounds (the group-permuted row layout means a whole
# 512-node group must fit the 32768-row int16 window)
G1MAX = 32255
G2MIN, G2MAX = 8704, 40959
G3MIN = 17920

XSLAB_T = 24                      # node tiles per x slab
PGROUP = 4                        # node tiles per PSUM bank / table write
CALLCOLS = 16                     # gather-call column cap (2048 descs; half
                                  # the SWDGE ring, so desc-gen of the next
                                  # call overlaps the prior call's transfer)
MSGCOLS = 32                      # window-batch column budget (msg tile)


def _row_of(n):
    """Node -> table row. Groups of PGROUP tiles are internally permuted
    partition-major (row = group_base + p*gn + tile_in_group) so each SBUF
    partition's rows are contiguous in DRAM -> 1KB write descriptors."""
    n = np.asarray(n, dtype=np.int64)
    t = n // 128
    p = n % 128
    g = t // PGROUP
    t0 = g * PGROUP
    gn = np.minimum(PGROUP, NT - t0)
    return 128 + 128 * (n >= MIDSPLIT) + t0 * 128 + p * gn + (t - t0)


def _plan(edge_index):
    """Host-side preprocessing: per-core gather grids (region-major),
    window/call metadata, and the unshard permutation + output scalings."""
    ei0 = np.asarray(edge_index[0], dtype=np.int64)
    ei1 = np.asarray(edge_index[1], dtype=np.int64)
    self_idx = np.arange(N_NODES, dtype=np.int64)
    row = np.concatenate([ei0, self_idx])
    col = np.concatenate([ei1, self_idx])

    deg = np.bincount(row, minlength=N_NODES)
    dis = deg.astype(np.float64) ** -0.5
    ssum = np.bincount(row, weights=dis[col], minlength=N_NODES)
    sb = (dis * ssum).astype(np.float32)

    # 3-way split of each dest's ascending cols: first k1 -> G1, middle k2
    # -> G2, last k3 -> G3; clip keeps every part region-valid.
    n1o = np.bincount(row[col < G2MIN], minlength=N_NODES)
    n3o = np.bincount(row[col > G2MAX], minlength=N_NODES)
    m1 = np.bincount(row[col <= G1MAX], minlength=N_NODES)
    m3 = np.bincount(row[col >= G3MIN], minlength=N_NODES)
    k1 = np.clip((deg + 2) // 3, n1o, m1)
    k3 = np.clip((deg - k1 + 1) // 2, n3o, np.minimum(m3, deg - k1))
    k2 = deg - k1 - k3
    k = np.stack([k1, k2, k3])
    roff = np.stack([np.zeros_like(k1), k1, k1 + k2])
    maxk = k.max(axis=0)

    order = np.lexsort((col, row))
    rows_sorted = _row_of(col[order])
    starts = np.zeros(N_NODES + 1, dtype=np.int64)
    np.cumsum(deg, out=starts[1:])

    perms = []
    kw = np.zeros((3, N_CORES, WPC), dtype=np.int64)
    for c in range(N_CORES):
        sl = slice(c * DPC, (c + 1) * DPC)
        perm = np.lexsort((-deg[sl], -maxk[sl]))
        perms.append(perm)
        for r in range(3):
            ks = np.pad(k[r, sl][perm], (0, WPC * 128 - DPC))
            kw[r, c] = ks.reshape(WPC, 128).max(axis=1)
    cks = kw.max(axis=1)              # [3, WPC] global per-window maxima
    cks = np.maximum(cks, 1)

    offs = np.zeros((3, WPC), dtype=np.int64)   # col offset within segment
    batches = []                      # [3][nbatch] -> list of windows
    for r in range(3):
        offs[r] = np.concatenate([[0], np.cumsum(cks[r])[:-1]])
        bs, cur, acc = [], [], 0
        for w in range(WPC):
            kc = int(cks[r, w])
            if cur and acc + kc > MSGCOLS:
                bs.append(cur)
                cur, acc = [], 0
            cur.append(w)
            acc += kc
        if cur:
            bs.append(cur)
        batches.append(bs)
    Ls = cks.sum(axis=1)
    seg0 = np.concatenate([[0], np.cumsum(Ls)[:-1]])
    totc = int(Ls.sum())

    idx16, gdests = [], []
    grids = []
    for c in range(N_CORES):
        perm = perms[c]
        gdest = np.full((WPC, 128), -1, dtype=np.int64)
        dest_all = np.full(WPC * 128, -1, dtype=np.int64)
        dest_all[:DPC] = c * DPC + perm
        gdest[:] = dest_all.reshape(WPC, 128)
        valid = dest_all >= 0
        d = dest_all[valid]
        p = (np.arange(WPC * 128) % 128)[valid]
        wi = (np.arange(WPC * 128) // 128)[valid]
        subs = []
        for r in range(3):
            sub = np.full((128, int(Ls[r])), GPAD[r], dtype=np.int16)
            kk = k[r, d]
            tot = int(kk.sum())
            if tot:
                intra = np.arange(tot) - np.repeat(
                    np.concatenate([[0], np.cumsum(kk)[:-1]]), kk)
                tgt = np.repeat(p * Ls[r] + offs[r, wi], kk) + intra
                src = np.repeat(starts[d] + roff[r, d], kk) + intra
                vals = rows_sorted[src] - GBASE[r]
                assert vals.min() >= 0 and vals.max() <= 32767, (
                    r, vals.min(), vals.max())
                sub.ravel()[tgt] = vals.astype(np.int16)
            subs.append(sub)
        grid = np.concatenate(subs, axis=1)
        grids.append(grid)
        # dma_gather idx layout: slot j (= col*128 + part) at
        # [16g + j%16, j//16] for each of 8 replicated 16-row groups.
        L = grid.T.ravel()
        base = L.reshape(totc * 8, 16).T
        idx16.append(np.ascontiguousarray(np.tile(base, (8, 1))))
        gdests.append(gdest)

    # per-call max table row read (over all cores -> one shared NEFF);
    # the sliced gather operand makes Tile start the call as soon as the
    # matmul has written that prefix of the table.
    maxrows = []
    for r in range(3):
        mr = []
        for batch in batches[r]:
            o = int(seg0[r] + offs[r, batch[0]])
            bc = int(sum(cks[r, w] for w in batch))
            m = 0
            for c in range(N_CORES):
                m = max(m, int(grids[c][:, o:o + bc].max()))
            mr.append(GBASE[r] + m + 1)
        maxrows.append(mr)

    return {
        "dis": dis.astype(np.float32),
        "sb": sb,
        "cks": cks,
        "offs": offs,
        "seg0": seg0,
        "totc": totc,
        "batches": batches,
        "maxrows": maxrows,
        "idx16": idx16,
        "gdest": gdests,
    }


def _build_bass(cks, offs, seg0, totc, batches, maxrows):
    import concourse.bacc as bacc
    import concourse.mybir as mybir
    import concourse.tile as tile
    from concourse.library_config import mlp

    fp32 = mybir.dt.float32
    fp16 = mybir.dt.float16
    i16 = mybir.dt.int16

    nc = bacc.Bacc(
        "TRN2",
        target_bir_lowering=False,
        dynamic_dma_scratch_size=65536,
        num_swdge_queues=4,
    )

    xT = nc.dram_tensor("xT", [IN_CH, NPAD], fp16, kind="ExternalInput")
    wT = nc.dram_tensor("wT", [IN_CH, OUT_CH], fp16, kind="ExternalInput")
    idx = nc.dram_tensor("idx", [128, totc * 8], i16, kind="ExternalInput")

    gtab = nc.dram_tensor("gtab", [GT_ROWS, OUT_CH], fp16, kind="Internal")
    outd = nc.dram_tensor("outd", [WPC, 128, OUT_CH], fp32,
                          kind="ExternalOutput")

    with tile.TileContext(nc) as tc:
        # phase-B pools open first: their SBUF lives below the phase-A
        # pools, so the first gathers don't inherit a WAR dependency on
        # phase A's last instructions through memory reuse.
        with (
            tc.tile_pool(name="constB", bufs=1) as bpool,
            tc.tile_pool(name="msg", bufs=6) as mpool,
            tc.tile_pool(name="accp", bufs=1) as apool,
            tc.tile_pool(name="red", bufs=4) as rpool,
        ):
            nc.gpsimd.load_library(mlp)
            ix = bpool.tile([128, totc * 8], i16, tag="ix")
            nc.scalar.dma_start(ix[:], idx[:])

            # ------------- phase A: g = xs @ W.T (pre-scaled x) -------------
            with (
                tc.tile_pool(name="constA", bufs=1) as cpool,
                tc.tile_pool(name="xslab", bufs=2) as xpool,
                tc.tile_pool(name="gout", bufs=4) as gpool,
                tc.tile_pool(name="psum", bufs=2, space="PSUM") as ppool,
            ):
                wt0 = cpool.tile([128, OUT_CH], fp16, tag="wt0")
                wt1 = cpool.tile([128, OUT_CH], fp16, tag="wt1")
                nc.sync.dma_start(wt0[:], wT[0:128, :])
                nc.sync.dma_start(wt1[:], wT[128:256, :])

                # zero bands absorb padding gathers
                zt = cpool.tile([128, OUT_CH], fp16, tag="zt")
                nc.vector.memset(zt[:], 0.0)
                for r0 in (0, 24704, 50304):
                    nc.scalar.dma_start(gtab[r0:r0 + 128, :], zt[:])

                nslab = (NT + XSLAB_T - 1) // XSLAB_T
                for s in range(nslab):
                    t0 = s * XSLAB_T
                    ntile = min(XSLAB_T, NT - t0)
                    cols = ntile * 128
                    xa = xpool.tile([128, XSLAB_T * 128], fp16, tag="xa")
                    xb = xpool.tile([128, XSLAB_T * 128], fp16, tag="xb")
                    c0 = t0 * 128
                    nc.sync.dma_start(xa[:, 0:cols], xT[0:128, c0:c0 + cols])
                    nc.sync.dma_start(xb[:, 0:cols], xT[128:256, c0:c0 + cols])
                    t = 0
                    while t < ntile:
                        gn = min(PGROUP, ntile - t)
                        # separate PSUM tiles: accumulation groups may not
                        # interleave within one bank (only the last start/
                        # stop pair survives), but do across banks.
                        pss = []
                        for j in range(gn):
                            psj = ppool.tile([128, OUT_CH], fp32,
                                             tag=f"ps{j}", name=f"ps_{j}")
                            pss.append(psj)
                        gtl = gpool.tile([128, PGROUP, OUT_CH], fp16, tag="gt")
                        for j in range(gn):
                            sl = slice((t + j) * 128, (t + j + 1) * 128)
                            nc.tensor.matmul(
                                pss[j][:], xa[:, sl], wt0[:],
                                start=True, stop=False,
                            )
                        for j in range(gn):
                            sl = slice((t + j) * 128, (t + j + 1) * 128)
                            nc.tensor.matmul(
                                pss[j][:], xb[:, sl], wt1[:],
                                start=False, stop=True,
                            )
                        for j in range(gn):
                            nc.vector.tensor_scalar_mul(
                                gtl[:, j, :], pss[j][:], 1.0
                            )
                        node0 = (t0 + t) * 128
                        w0 = node0 + 128 + 128 * (node0 >= MIDSPLIT)
                        dst = gtab[w0:w0 + gn * 128, :].rearrange(
                            "(p a) c -> p a c", a=gn)
                        nc.scalar.dma_start(dst, gtl[:, 0:gn, :])
                        t += gn

            # ------------- phase B: gather + segment reduce -----------------
            if True:
                acc = apool.tile([128, WPC, OUT_CH], fp32, tag="acc")
                gq = 0
                for r in range(3):
                    # fire calls in table-readiness order
                    order = np.argsort(maxrows[r], kind="stable")
                    for bi in order:
                        batch = batches[r][bi]
                        bc = int(sum(cks[r, w] for w in batch))
                        o = int(seg0[r] + offs[r, batch[0]])
                        src = gtab[GBASE[r]:maxrows[r][bi], :]
                        msg = mpool.tile([128, MSGCOLS, OUT_CH], fp16,
                                         tag="msg")
                        for cc0 in range(0, bc, CALLCOLS):
                            cc = min(CALLCOLS, bc - cc0)
                            nc.gpsimd.dma_gather(
                                msg[:, cc0:cc0 + cc, :], src,
                                ix[:, (o + cc0) * 8:(o + cc0 + cc) * 8],
                                128 * cc, 128 * cc, OUT_CH,
                                queue_num=gq % 4,
                                single_packet=False,
                            )
                            gq += 1
                        add = mybir.AluOpType.add
                        coff = 0
                        for w in batch:
                            kc = int(cks[r, w])
                            o2 = coff
                            kk = kc
                            # contiguous in-place fp16 fold tree (DVE runs
                            # ~4x faster than a strided tensor_reduce)
                            while kk > 2:
                                f = kk // 2
                                nc.vector.tensor_tensor(
                                    msg[:, o2:o2 + f, :],
                                    msg[:, o2:o2 + f, :],
                                    msg[:, o2 + kk - f:o2 + kk, :], op=add,
                                )
                                kk -= f
                            if r == 0:
                                if kk == 2:
                                    nc.vector.tensor_tensor(
                                        acc[:, w:w + 1, :],
                                        msg[:, o2:o2 + 1, :],
                                        msg[:, o2 + 1:o2 + 2, :], op=add,
                                    )
                                else:
                                    nc.vector.tensor_scalar_mul(
                                        acc[:, w:w + 1, :],
                                        msg[:, o2:o2 + 1, :], 1.0,
                                    )
                            else:
                                rt = rpool.tile([128, OUT_CH], fp32, tag="rt")
                                if kk == 2:
                                    nc.vector.tensor_tensor(
                                        rt[:], msg[:, o2:o2 + 1, :],
                                        msg[:, o2 + 1:o2 + 2, :], op=add,
                                    )
                                else:
                                    nc.vector.tensor_scalar_mul(
                                        rt[:], msg[:, o2:o2 + 1, :], 1.0,
                                    )
                                nc.vector.tensor_tensor(
                                    acc[:, w, :], acc[:, w, :], rt[:], op=add,
                                )
                                if r == 2:
                                    nc.sync.dma_start(outd[w], acc[:, w, :])
                            coff += kc

    nc.compile()
    return nc


def _install_ntff_shim():
    """The agent image's antenv lacks axon_hooks; register a shim wired to
    the libaxon NTFF profiler so trace=True works."""
    import sys
    import types
    try:
        import antenv.axon_hooks  # noqa: F401
        return
    except ImportError:
        pass
    hook = None
    try:
        from trn_agent_boot.trn_boot import _ntff_profile_via_ctypes
        hook = _ntff_profile_via_ctypes("/opt/axon/libaxon_pjrt.so")
    except Exception:
        hook = None
    mod = types.ModuleType("antenv.axon_hooks")
    mod._hook = hook
    mod.get_axon_ntff_profile_hook = lambda: mod._hook
    def _set(h):
        mod._hook = h
    mod.set_axon_ntff_profile_hook = _set
    sys.modules["antenv.axon_hooks"] = mod
    try:
        import antenv
        antenv.axon_hooks = mod
    except Exception:
        pass


def kernel(x, edge_index, W, b):
    import os
    os.environ.setdefault("NEURON_RT_RESET_CORES", "1")
    x = np.asarray(x, dtype=np.float32)
    W = np.asarray(W, dtype=np.float32)
    b = np.asarray(b, dtype=np.float32)

    plan = _plan(edge_index)
    nc = _build_bass(plan["cks"], plan["offs"], plan["seg0"], plan["totc"],
                     plan["batches"], plan["maxrows"])

    dis = plan["dis"]
    xs = x * dis[:, None]
    xT_pad = np.zeros((IN_CH, NPAD), dtype=np.float16)
    xT_pad[:, :N_NODES] = xs.T.astype(np.float16)
    wT = np.ascontiguousarray(W.T.astype(np.float16))

    in_maps = []
    for c in range(N_CORES):
        in_maps.append({
            "xT": xT_pad,
            "wT": wT,
            "idx": plan["idx16"][c],
        })

    _install_ntff_shim()
    from concourse.bass_utils import run_bass_kernel_spmd
    res = run_bass_kernel_spmd(nc, in_maps, core_ids=list(range(N_CORES)))
    globals()["_last_results"] = res

    out = np.empty((N_NODES, OUT_CH), dtype=np.float32)
    for c in range(N_CORES):
        outd = res.results[c]["outd"]
        gdest = plan["gdest"][c]
        mask = gdest >= 0
        out[gdest[mask]] = outd[mask]
    out *= dis[:, None]
    out += plan["sb"][:, None] * b[None, :]
    return out
